# revision 12
# baseline (speedup 1.0000x reference)
"""DGCNN-style graph conv kernel for Trainium2 (8 NeuronCores, data-parallel over batch).

Reference computation (per sample):
  idx = knn(xyz, 20)                        # top-20 by -||xi-xj||^2, per point
  geo = relu(BN1(w1 @ [nb_xyz - xyz; xyz]))
  fea = relu(BN2(w2 @ [nb_feat - feat; feat]))
  out = max_k concat([geo, fea])            # (128, N)

Algebraic collapse (relu/max commute, BN scale > 0):
  out[c, n] = relu( max_k (G[c, idx[n, k]] + hb[c]) + H[c, n] )
  G + hb = s * (Wa @ X) + hb  (neighbor part incl. bias; k=0 is self)
  H = s * ((Wb - Wa) @ X)     (center part)
  hb = s * b + shift          (folded BN bias; constant per channel so it
                               commutes with the max and rides the G matmuls)
where for c < 64: Wa/Wb from w1, X = xyz; for c >= 64: from w2, X = feat.

Device pipeline per core (1 sample), points in the partition dim throughout
(the output transpose to (C, N) happens on the host for free):
  1. GB^T[n, c] = G^T + hb: per-128-point-chunk matmuls -> SBUF (self slot)
     and HBM fp32 rows (gather source, 512B contiguous per point)
  2. H^T[n, c] in SBUF fp32 (same matmul structure)
  3. D-chunk (128 rows x 2048) = -(dist^2) via one K=5 augmented fp32 matmul
  4. top-20 per row (self lands in slot 0): 3 rounds of
     (max8, max_index8, match_replace8), indices in uint32
  5. per chunk, 19 indirect DMAs (standard DynamicAP InstDMACopy on the
     Pool dynamic queue, ~1.1us each) gather ag[point, slot, c] straight
     from the top-k index columns -- no index reshuffle needed.  The
     swdge dma_gather ucode would be ~4x cheaper per row but crashes this
     runtime whenever a kernel issues more than one.
  6. Vector: tensor_reduce max over the 19 slots (strided axis), max with
     the chunk's own GB^T rows (self), + H^T, relu; out^T -> DRAM.
"""
import numpy as np

B, N, C, K = 8, 2048, 128, 20
KG = K - 1           # 19 gathered neighbor slots (self via elementwise max)
H2 = C // 2          # 64
EPS = 1e-5
NEG = -3.0e38
NCHUNK = N // 128    # 16 topk chunks

_compiled = None


def _build():
    import concourse.bass as bass
    import concourse.bacc as bacc
    import concourse.mybir as mybir
    import concourse.tile as tile

    f32 = mybir.dt.float32
    bf16 = mybir.dt.bfloat16
    u32 = mybir.dt.uint32

    nc = bacc.Bacc("TRN2")
    xyz_in = nc.declare_dram_parameter("xyz", [3, N], f32, isOutput=False)
    feat_in = nc.declare_dram_parameter("feat", [C, N], f32, isOutput=False)
    wg_xyz_in = nc.declare_dram_parameter("wg_xyz", [3, H2], f32, isOutput=False)
    wg_feat_in = nc.declare_dram_parameter("wg_feat", [C, H2], f32, isOutput=False)
    wh_xyz_in = nc.declare_dram_parameter("wh_xyz", [3, H2], f32, isOutput=False)
    wh_feat_in = nc.declare_dram_parameter("wh_feat", [C, H2], f32, isOutput=False)
    hb_row_in = nc.declare_dram_parameter("hb_row", [1, C], f32, isOutput=False)
    out_dram = nc.declare_dram_parameter("out_t", [N, C], f32, isOutput=True)

    # G^T + hb rows in HBM: gather source, 256B bf16 contiguous per point
    gt_dram = nc.dram_tensor("gt_scratch", [N, C], bf16)

    with tile.TileContext(nc) as tc:
        with (
            tc.tile_pool(name="const", bufs=1) as cpool,
            tc.tile_pool(name="work", bufs=2) as wpool,
            tc.tile_pool(name="ag", bufs=2) as agpool,
            tc.tile_pool(name="agbuf", bufs=5) as abpool,
            tc.tile_pool(name="psum", bufs=2, space="PSUM") as ppool,
        ):
            xyz_t = cpool.tile([3, N], f32)
            feat_t = cpool.tile([C, N], f32)
            wgf_t = cpool.tile([C, H2], f32)
            whx_t = cpool.tile([3, H2], f32)
            whf_t = cpool.tile([C, H2], f32)
            hbr_t = cpool.tile([1, C], f32)
            nc.sync.dma_start(xyz_t[:], xyz_in[:])
            nc.sync.dma_start(feat_t[:], feat_in[:])
            nc.sync.dma_start(wgf_t[:], wg_feat_in[:])
            nc.sync.dma_start(whx_t[:], wh_xyz_in[:])
            nc.sync.dma_start(whf_t[:], wh_feat_in[:])
            nc.sync.dma_start(hbr_t[:], hb_row_in[:])

            # ---- xx[n] = sum_d xyz[d,n]^2 ----
            sq_t = cpool.tile([3, N], f32)
            nc.vector.tensor_tensor(
                out=sq_t[:], in0=xyz_t[:], in1=xyz_t[:], op=mybir.AluOpType.mult
            )
            ones3_t = cpool.tile([3, 1], f32)
            nc.vector.memset(ones3_t[:], 1.0)
            xx_slot = ppool.tile([128, N], f32, space="PSUM", tag="d")
            for j in range(4):
                nc.tensor.matmul(
                    out=xx_slot[0:1, 512 * j:512 * (j + 1)],
                    lhsT=ones3_t[:],
                    rhs=sq_t[:, 512 * j:512 * (j + 1)],
                    start=True, stop=True,
                )
            xx_t = cpool.tile([1, N], f32)
            nc.scalar.copy(xx_t[:], xx_slot[0:1, :])

            # ---- lhs5 = [xyz; 1; xx], rhs5 = [2 xyz; -xx; -1] ----
            # (row order chosen so lhs5[0:4] doubles as the [xyz; 1] lhsT of
            # the GB^T bias-carrying matmul). compute-engine ops must start
            # at quadrant-aligned partitions, so rows 3/4 go in via DMA.
            lhs5 = cpool.tile([5, N], f32)
            rhs5 = cpool.tile([5, N], f32)
            ones_row = cpool.tile([1, N], f32)
            neg1_row = cpool.tile([1, N], f32)
            nxx_t = cpool.tile([1, N], f32)
            nc.vector.memset(ones_row[:], 1.0)
            nc.vector.memset(neg1_row[:], -1.0)
            nc.vector.tensor_scalar_mul(nxx_t[:], xx_t[:], -1.0)
            nc.vector.tensor_copy(lhs5[0:3, :], xyz_t[:])
            nc.vector.tensor_scalar_mul(rhs5[0:3, :], xyz_t[:], 2.0)
            nc.sync.dma_start(lhs5[3:4, :], ones_row[:])
            nc.sync.dma_start(lhs5[4:5, :], xx_t[:])
            nc.sync.dma_start(rhs5[3:4, :], nxx_t[:])
            nc.sync.dma_start(rhs5[4:5, :], neg1_row[:])

            # geo-part rhs of GB^T: [wgx; hb[0:64]] (4, 64)
            rhs4g = cpool.tile([4, H2], f32)
            nc.sync.dma_start(rhs4g[0:3, :], wg_xyz_in[:])
            nc.sync.dma_start(rhs4g[3:4, :], hb_row_in[:, 0:H2])

            # ---- GB^T chunk: (128 points, 128 ch) bf16 -> SBUF + HBM ----
            gts_t = cpool.tile([128, N], bf16)  # [point, 16 chunks x 128 ch]

            def emit_gt(c):
                ps = slice(128 * c, 128 * (c + 1))
                slot = ppool.tile([128, N], f32, space="PSUM", tag="d")
                # cols 0:64 = [xyz;1]^T [wgx; hb_g]
                nc.tensor.matmul(out=slot[:, 0:H2], lhsT=lhs5[0:4, ps],
                                 rhs=rhs4g[:], start=True, stop=True)
                # cols 64:128 = 1^T hb_f  +  feat^T wgf (PSUM accumulate)
                nc.tensor.matmul(out=slot[:, H2:C], lhsT=ones_row[:, ps],
                                 rhs=hbr_t[:, H2:C], start=True, stop=False)
                nc.tensor.matmul(out=slot[:, H2:C], lhsT=feat_t[:, ps],
                                 rhs=wgf_t[:], start=False, stop=True)
                nc.scalar.copy(gts_t[:, ps], slot[:, 0:C])
                nc.sync.dma_start(gt_dram[ps, :], gts_t[:, ps])

            # ---- H^T chunk: (128 points, 128 ch) fp32 in SBUF ----
            ht_t = cpool.tile([128, N], f32)

            def emit_ht(c):
                ps = slice(128 * c, 128 * (c + 1))
                slot = ppool.tile([128, N], f32, space="PSUM", tag="d")
                nc.tensor.matmul(out=slot[:, 0:H2], lhsT=lhs5[0:3, ps],
                                 rhs=whx_t[:], start=True, stop=True)
                nc.tensor.matmul(out=slot[:, H2:C], lhsT=feat_t[:, ps],
                                 rhs=whf_t[:], start=True, stop=True)
                nc.scalar.copy(ht_t[:, ps], slot[:, 0:C])

            # ---- per-chunk: D matmul + top-20 ----
            idxs_tiles = {}

            def emit_chunk(c):
                d_ps = ppool.tile([128, N], f32, space="PSUM", tag="d")
                for j in range(4):
                    fs = slice(512 * j, 512 * (j + 1))
                    nc.tensor.matmul(
                        out=d_ps[:, fs],
                        lhsT=lhs5[:, 128 * c:128 * (c + 1)],
                        rhs=rhs5[:, fs],
                        start=True, stop=True,
                    )
                d_sb = wpool.tile([128, N], f32, tag="dsb")
                nc.scalar.copy(d_sb[:], d_ps[:])
                vals = wpool.tile([128, 24], f32, tag="vals")
                idxs = wpool.tile([128, 24], u32, tag="idxs")
                for r in range(3):
                    v8 = vals[:, 8 * r:8 * (r + 1)]
                    i8 = idxs[:, 8 * r:8 * (r + 1)]
                    nc.vector.max(out=v8, in_=d_sb[:])
                    nc.vector.max_index(out=i8, in_max=v8, in_values=d_sb[:])
                    if r < 2:
                        nc.vector.match_replace(
                            out=d_sb[:], in_to_replace=v8, in_values=d_sb[:],
                            imm_value=NEG,
                        )
                idxs_tiles[c] = idxs

            # ---- gather: 19 indirect DMAs per chunk (slots 1..19) ----
            ag_tiles = {}

            def emit_gather(c):
                idxs = idxs_tiles.pop(c)
                ag = abpool.tile([128, KG, C], bf16, tag="ag")
                for j in range(KG):
                    nc.gpsimd.indirect_dma_start(
                        out=ag[:, j, :], out_offset=None, in_=gt_dram[:],
                        in_offset=bass.IndirectOffsetOnAxis(
                            ap=idxs[:, 1 + j:2 + j], axis=0),
                    )
                ag_tiles[c] = ag

            # ---- finish: reduce over 19 + self + H^T + relu -> out ----
            def emit_finish(c):
                ag = ag_tiles.pop(c)
                cs = slice(128 * c, 128 * (c + 1))
                mx = mybir.AluOpType.max
                agf = ag[:].rearrange("p q c -> p (q c)")
                t1 = agpool.tile([128, 8 * C], bf16, tag="t1")
                nc.vector.tensor_tensor(
                    out=t1[:], in0=agf[:, 0:8 * C], in1=agf[:, 8 * C:16 * C],
                    op=mx)
                nc.vector.tensor_tensor(
                    out=t1[:, 0:4 * C], in0=t1[:, 0:4 * C],
                    in1=t1[:, 4 * C:8 * C], op=mx)
                nc.vector.tensor_tensor(
                    out=t1[:, 0:2 * C], in0=t1[:, 0:2 * C],
                    in1=t1[:, 2 * C:4 * C], op=mx)
                # fold slots 16..18 + self into the last two lanes
                nc.vector.tensor_tensor(
                    out=t1[:, 2 * C:4 * C], in0=agf[:, 16 * C:18 * C],
                    in1=ag[:, 17:19, :].rearrange("p q c -> p (q c)"), op=mx)
                nc.vector.tensor_tensor(
                    out=t1[:, 4 * C:5 * C], in0=t1[:, 2 * C:3 * C],
                    in1=gts_t[:, cs], op=mx)
                nc.vector.tensor_tensor(
                    out=t1[:, 0:2 * C], in0=t1[:, 0:2 * C],
                    in1=t1[:, 3 * C:5 * C], op=mx)
                s_t = agpool.tile([128, C], f32, tag="s")
                nc.vector.tensor_tensor(
                    out=s_t[:], in0=t1[:, 0:C], in1=t1[:, C:2 * C], op=mx)
                t_t = agpool.tile([128, C], f32, tag="t")
                nc.vector.tensor_add(t_t[:], s_t[:], ht_t[:, cs])
                o_t = agpool.tile([128, C], f32, tag="o")
                nc.vector.tensor_scalar_max(o_t[:], t_t[:], 0.0)
                nc.sync.dma_start(out_dram[cs, :], o_t[:])

            # Emission: chunk 0's D/top-k first, then all GB^T chunks
            # (gathers need every row); per-chunk gathers start right after
            # their top-k, finishes two chunks behind (the Pool dynamic
            # queue at ~21us/chunk outpaces Vector, so finishes trail).
            for c in range(8):
                emit_gt(c)
            emit_chunk(0)
            for c in range(8, NCHUNK):
                emit_gt(c)
            emit_ht(0)
            for c in range(1, NCHUNK + 1):
                if c < NCHUNK:
                    emit_chunk(c)
                    emit_ht(c)
                emit_gather(c - 1)
                if c >= 2:
                    emit_finish(c - 2)
            emit_finish(NCHUNK - 1)

    nc.compile()
    return nc


def _fold_params(w1, b1, g1, be1, m1, v1, w2, b2, g2, be2, m2, v2):
    s1 = g1 / np.sqrt(v1 + EPS)
    sh1 = be1 - m1 * s1
    s2 = g2 / np.sqrt(v2 + EPS)
    sh2 = be2 - m2 * s2
    wg_xyz = (s1[None, :] * w1[:, 0:3].T).astype(np.float32)        # (3, 64)
    wh_xyz = (s1[None, :] * (w1[:, 3:6] - w1[:, 0:3]).T).astype(np.float32)
    wg_feat = (s2[None, :] * w2[:, 0:C].T).astype(np.float32)       # (128, 64)
    wh_feat = (s2[None, :] * (w2[:, C:2 * C] - w2[:, 0:C]).T).astype(np.float32)
    hb = np.concatenate([s1 * b1 + sh1, s2 * b2 + sh2]).astype(np.float32)[:, None]
    return wg_xyz, wg_feat, wh_xyz, wh_feat, hb


def _in_maps(xyz, features, wg_xyz, wg_feat, wh_xyz, wh_feat, hb):
    hb_row = np.ascontiguousarray(hb.reshape(1, C)).astype(np.float32)
    in_maps = []
    for bb in range(B):
        in_maps.append({
            "xyz": np.ascontiguousarray(xyz[bb]),
            "feat": np.ascontiguousarray(features[bb]),
            "wg_xyz": wg_xyz, "wg_feat": wg_feat,
            "wh_xyz": wh_xyz, "wh_feat": wh_feat,
            "hb_row": hb_row,
        })
    return in_maps


def kernel(xyz, features, w1, b1, g1, be1, m1, v1, w2, b2, g2, be2, m2, v2, k):
    global _compiled
    assert int(k) == K
    from concourse.bass_utils import run_bass_kernel_spmd

    if _compiled is None:
        _compiled = _build()
    nc = _compiled

    folded = _fold_params(
        np.asarray(w1), np.asarray(b1), np.asarray(g1), np.asarray(be1),
        np.asarray(m1), np.asarray(v1), np.asarray(w2), np.asarray(b2),
        np.asarray(g2), np.asarray(be2), np.asarray(m2), np.asarray(v2),
    )
    xyz = np.ascontiguousarray(np.asarray(xyz, dtype=np.float32))
    features = np.ascontiguousarray(np.asarray(features, dtype=np.float32))

    in_maps = _in_maps(xyz, features, *folded)
    res = run_bass_kernel_spmd(nc, in_maps, list(range(B)))
    out = np.stack(
        [res.results[bb]["out_t"].T for bb in range(B)], axis=0)
    return np.ascontiguousarray(out).astype(np.float32)
